# revision 2
# baseline (speedup 1.0000x reference)
"""KeyedSensor encrypt->decrypt roundtrip kernel for Trainium2 (8 NeuronCores).

The reference computes
    cipher[:, j] = h[:, invperm[j]] * scale[invperm[j]]
    h_rec[:, i]  = cipher[:, perm[i]] / scale[i]
with invperm = argsort(perm), so invperm[perm[i]] = i and
    h_rec[:, i] = (h[:, i] * scale[i]) / scale[i]  == h[:, i]
exactly (up to two fp32 roundings, rel err <= ~1.2e-7). The permutation
cancels identically for ANY permutation and any nonzero scale, so the
kernel is a data-parallel copy of x: each of the 8 cores copies its
32-row shard of x (32 x 196608 f32, ~25 MiB) HBM->HBM.
"""

import sys

for _p in ("/opt/trn_rl_repo",):
    if _p not in sys.path:
        sys.path.insert(0, _p)

import numpy as np

import concourse.bass as bass
import concourse.mybir as mybir
from concourse.bass_utils import run_bass_kernel_spmd

N = 256
C, H, W = 3, 256, 256
D = C * H * W  # 196608
NCORES = 8
ROWS = N // NCORES  # 32 rows per core
_nc_cache = None


def build_nc():
    """Per-core Bass kernel: copy x_shard (ROWS, D) -> y_shard (ROWS, D).

    Two DRAM->DRAM DMAs, one per HWDGE ring (sync=SP, scalar=ACT), so both
    descriptor rings fill in parallel and all 16 SDMA engines ramp together.
    Measured ~86.6 us/core steady (25.2 MB payload, ~330 GB/s, HBM-bound).
    """
    nc = bass.Bass()
    x = nc.declare_dram_parameter("x", [ROWS, D], mybir.dt.float32, isOutput=False)
    y = nc.declare_dram_parameter("y", [ROWS, D], mybir.dt.float32, isOutput=True)

    half = ROWS // 2
    with nc.Block() as block, nc.semaphore("dma_sem") as dma_sem:

        @block.scalar
        def _(scalar):
            scalar.dma_start(out=y[half:, :], in_=x[half:, :]).then_inc(dma_sem, 16)

        @block.sync
        def _(sync):
            sync.dma_start(out=y[:half, :], in_=x[:half, :]).then_inc(dma_sem, 16)
            sync.wait_ge(dma_sem, 32)

    return nc


def _get_nc():
    global _nc_cache
    if _nc_cache is None:
        _nc_cache = build_nc()
    return _nc_cache


def make_in_maps(x_flat):
    return [{"x": x_flat[i * ROWS : (i + 1) * ROWS]} for i in range(NCORES)]


def kernel(x, perm=None, scale=None, **_):
    x = np.asarray(x, dtype=np.float32)
    x_flat = np.ascontiguousarray(x.reshape(N, D))
    nc = _get_nc()
    res = run_bass_kernel_spmd(nc, make_in_maps(x_flat), list(range(NCORES))).results
    out = np.concatenate([r["y"] for r in res], axis=0)
    return out.reshape(N, C, H, W)
